# revision 12
# baseline (speedup 1.0000x reference)
"""Trainium2 Bass kernel for nn_Listener (LSTM listener + dense encoders).

Reference computation (per full batch B=512):
    emb = embed_table[message]                       # [B, T, 512]
    LSTM over T=128 steps, HIDDEN=1024:
        gated = [x_t, h] @ W_cell + b_cell           # [B, 4096] (i, g, f, o)
        f = sigmoid(f + 1); c = f*c + sigmoid(i)*tanh(g); h = sigmoid(o)*tanh(c)
    images_encoded = images @ W_img + b_img          # [B, 1024]
    hidden_encoded = h @ W_hid + b_hid               # [B, 1024]
    returns (images_encoded, hidden_encoded)

Strategy (8 NeuronCores, data-parallel over batch, 64 rows/core):
  * Embedding lookup + x-projection folded into one bf16 table:
        M2 = embed_table @ W_cell[:512] + b_cell   (gathered per token)
    injected into PSUM with a k=128 selection matmul that also seeds the
    accumulation groups (start=True).  The f-gate +1.0 is applied as an
    exact activation bias instead of being folded into the table.
  * Per-core batch 64 = half the PSUM partitions; hidden units split in
    half across partition ranges (partitions 0:64 <-> units u, 64:128 <->
    units u+512).  Recurrence matmul pairs (M=64) run concurrently in the
    two PE column groups -> full 128x128 bf16 utilization (216ns/pair).
  * The per-step critical path (gates -> c -> h -> h^T -> next matmul) is
    software-pipelined: every gate bank's matmuls are split into two
    256-column chunks so activations start as soon as each half-bank
    completes; every elementwise op is chunked with separate tiles
    (tile-granular deps); cmul/h run on the gpsimd engine; transposes are
    emitted per-h-chunk; next step's matmuls begin as soon as the low
    half of h^T is cast.  Bank order (f, g, i, o) gives the long
    c-chain (sigf->cmul->cnew->tanhc) a head start.
  * PSUM: 8 banks: gp_f, gp_g, gp_i, gp_oa, gp_ob, tp_lo, tp_hi, oip.
    The o-gate uses two half-banks so sigmoid(o) chunk a never collides
    with PE writes of chunk b.  The hidden-encoder accumulator reuses
    gp_f's bank via the tile-pool tag ring.
  * Encoder weights are DMA'd behind W_h during the recurrence; both
    encoders run on the PE right after the last step.
"""

import numpy as np

B, T = 512, 128
HIDDEN = 1024
VOCAB = 1024
EMB = 512
OUT = 1024
D_IMG = 2048
NCORES = 8
BS = B // NCORES  # 64 batch rows per core
HH = HIDDEN // 2  # 512 = per-half hidden units
HQ = HH // 2      # 256 = per-chunk columns

_CACHE = {}

# ci order: low blocks (h^T blocks 0,1) first, then high (blocks 2,3)
CI_ORDER = (0, 4, 1, 5, 2, 6, 3, 7)


def _build_nc(n_steps: int):
    import os
    # schedule_block_v2 ASAP scheduler: dependency-driven, respects emission
    # order for ready ties (the legacy CoreSim scheduler misorders the ACT
    # queue because its PE cost model lacks column-tile concurrency)
    os.environ.setdefault("TILE_SCHEDULER", "asap")
    import concourse.bass as bass
    import concourse.mybir as mybir
    from concourse import bacc, tile

    f32 = mybir.dt.float32
    f32r = mybir.dt.float32r
    bf16 = mybir.dt.bfloat16
    i32 = mybir.dt.int32
    AF = mybir.ActivationFunctionType

    nc = bacc.Bacc("TRN2", target_bir_lowering=False, debug=False)

    m2p_d = nc.declare_dram_parameter("m2p", [2 * VOCAB, HH * 4], bf16, isOutput=False)
    wh_d = nc.declare_dram_parameter("wh", [HIDDEN, 4 * HIDDEN], bf16, isOutput=False)
    msg2_d = nc.declare_dram_parameter("msg2", [2 * BS, T], i32, isOutput=False)
    sfull_d = nc.declare_dram_parameter("sfull", [2 * BS, 2 * BS], bf16, isOutput=False)
    ident_d = nc.declare_dram_parameter("ident", [128, 128], bf16, isOutput=False)
    identf_d = nc.declare_dram_parameter("identf", [128, 128], f32, isOutput=False)
    imgs_d = nc.declare_dram_parameter("imgs", [128, D_IMG // 2], f32, isOutput=False)
    wimg_d = nc.declare_dram_parameter("wimg", [D_IMG, OUT], bf16, isOutput=False)
    whid_d = nc.declare_dram_parameter("whid", [HIDDEN, OUT], bf16, isOutput=False)
    o2_d = nc.declare_dram_parameter("o2", [2, 128], f32r, isOutput=False)
    bimg2_d = nc.declare_dram_parameter("bimg2", [2, OUT // 2], f32r, isOutput=False)
    bhid2_d = nc.declare_dram_parameter("bhid2", [2, OUT // 2], f32r, isOutput=False)
    oimg_d = nc.declare_dram_parameter("oimg", [128, OUT // 2], f32, isOutput=True)
    ohid_d = nc.declare_dram_parameter("ohid", [128, OUT // 2], f32, isOutput=True)

    with tile.TileContext(nc) as tc:
        with (
            tc.tile_pool(name="wpool", bufs=1) as wpool,
            tc.tile_pool(name="const", bufs=1) as cpool,
            tc.tile_pool(name="xg", bufs=2) as xgpool,
            tc.tile_pool(name="state", bufs=2) as stpool,
            tc.tile_pool(name="act", bufs=1) as apool,
            tc.tile_pool(name="outs", bufs=1) as opool,
            tc.tile_pool(name="psum", bufs=1, space="PSUM") as pspool,
        ):
            # ---- constants / small inputs ----
            msg2 = cpool.tile([2 * BS, T], i32, tag="msg2")
            nc.sync.dma_start(msg2[:], msg2_d[:])
            sfull = cpool.tile([2 * BS, 2 * BS], bf16, tag="sfull")
            nc.sync.dma_start(sfull[:], sfull_d[:])
            ident = cpool.tile([128, 128], bf16, tag="ident")
            nc.sync.dma_start(ident[:], ident_d[:])
            identf = cpool.tile([128, 128], f32, tag="identf")
            nc.sync.dma_start(identf[:], identf_d[:])
            o2 = cpool.tile([2, 128], f32r, tag="o2")
            nc.sync.dma_start(o2[:], o2_d[:])
            bimg2 = cpool.tile([2, OUT // 2], f32r, tag="bimg2")
            nc.sync.dma_start(bimg2[:], bimg2_d[:])
            bhid2 = cpool.tile([2, OUT // 2], f32r, tag="bhid2")
            nc.sync.dma_start(bhid2[:], bhid2_d[:])
            imgs = cpool.tile([128, D_IMG // 2], f32, tag="imgs")
            nc.sync.dma_start(imgs[:], imgs_d[:])

            # ---- weights: W_h first (step-1 critical), then W_img, W_hid ----
            wh_sb = []
            for ci in range(8):
                wt = wpool.tile([128, 4 * HIDDEN], bf16, tag=f"wh{ci}")
                nc.sync.dma_start(wt[:], wh_d[128 * ci : 128 * (ci + 1), :])
                wh_sb.append(wt)
            wimg_sb = []
            for ci in range(16):
                wt = wpool.tile([128, OUT], bf16, tag=f"wimg{ci}")
                nc.sync.dma_start(wt[:], wimg_d[128 * ci : 128 * (ci + 1), :])
                wimg_sb.append(wt)
            whid_sb = []
            for ci in range(8):
                wt = wpool.tile([128, OUT], bf16, tag=f"whid{ci}")
                nc.sync.dma_start(wt[:], whid_d[128 * ci : 128 * (ci + 1), :])
                whid_sb.append(wt)

            # ---- PSUM layout: 8 banks, one full-bank tile per bank.
            # The i and o gates each get TWO banks (a/b column chunks in
            # separate banks) so their chunk activations can start while the
            # PE still writes the other chunk (engine-read + PE-write of the
            # same bank is fatal; tile deps are tile-granular).  The encoder
            # accumulators reuse gate banks after the loop via the tag ring.
            gp_f = pspool.tile([128, HH], f32, tag="gp_f")    # bank 0
            gp_g = pspool.tile([128, HH], f32, tag="gp_g")    # bank 1
            gp_ia = pspool.tile([128, HH], f32, tag="gp_ia")  # bank 2 (cols 0:256)
            gp_ib = pspool.tile([128, HH], f32, tag="gp_ib")  # bank 3 (cols 0:256)
            gp_oa = pspool.tile([128, HH], f32, tag="gp_oa")  # bank 4 (cols 0:256)
            gp_ob = pspool.tile([128, HH], f32, tag="gp_ob")  # bank 5 (cols 0:256)
            tp_lo = pspool.tile([128, 2 * HH], bf16, tag="tp_lo")  # bank 6
            tp_hi = pspool.tile([128, 2 * HH], bf16, tag="tp_hi")  # bank 7

            # ---- images transposed up front (also PE warmup) ----
            # imgs packed [128, 1024]: partitions 0:64 = batch x feats 0:1024,
            # 64:128 = batch x feats 1024:2048.
            imT = cpool.tile([128, D_IMG // 2], bf16, tag="imT")
            for half in range(2):
                tph = pspool.tile(
                    [128, HH], f32, tag=("gp_f", "gp_g")[half], name=f"tpim{half}"
                )
                for q in range(4):
                    qq = 4 * half + q
                    nc.tensor.transpose(
                        out=tph[:, 128 * q : 128 * (q + 1)],
                        in_=imgs[:, 128 * qq : 128 * (qq + 1)],
                        identity=identf[:],
                    )
                nc.vector.tensor_copy(imT[:, 512 * half : 512 * (half + 1)], tph[:])

            def imT_sl(ci):
                q, hi = (ci - 8, 64) if ci >= 8 else (ci, 0)
                return imT[:, 128 * q + hi : 128 * q + hi + 64]

            # ---- LSTM state init (c in two chunk tiles, double-buffered) ----
            c_prev = []
            for k in range(2):
                ct = stpool.tile([128, HQ], f32, tag=f"c{k}")
                nc.vector.memset(ct[:], 0.0)
                c_prev.append(ct)

            def gather(t):
                xg = xgpool.tile([2 * BS, 4 * HH], bf16, tag="xg", name=f"xg_{t}")
                nc.gpsimd.indirect_dma_start(
                    out=xg[:], out_offset=None, in_=m2p_d[:],
                    in_offset=bass.IndirectOffsetOnAxis(ap=msg2[:, t : t + 1], axis=0),
                )
                return xg

            def x_pass(xg, stop):
                # PSUM gate banks seeded with the gathered x-contribution
                for gp, c0, c1 in (
                    (gp_f, 2 * HH, 3 * HH),
                    (gp_g, HH, 2 * HH),
                    (gp_ia, 0, HQ),
                    (gp_ib, HQ, HH),
                    (gp_oa, 3 * HH, 3 * HH + HQ),
                    (gp_ob, 3 * HH + HQ, 4 * HH),
                ):
                    nc.tensor.matmul(
                        out=gp[:, 0 : c1 - c0], lhsT=sfull[:], rhs=xg[:, c0:c1],
                        start=True, stop=stop, skip_group_check=True,
                    )

            def hT_lhs(hT_lo, hT_hi, ci):
                q, hi = (ci - 4, 64) if ci >= 4 else (ci, 0)
                src = hT_lo if q < 2 else hT_hi
                return src[:, 128 * (q % 2) + hi : 128 * (q % 2) + hi + 64]

            xg_t = gather(0)

            hT_lo_prev = hT_hi_prev = h_prev = None

            for t in range(n_steps):
                last = t == n_steps - 1
                first = t == 0
                # gather for step t+1 (gpsimd queue head; xg double-buffered)
                if not last:
                    xg_n = gather(t + 1)

                # ---- PE stream for step t: the X seed matmul of each gate
                # bank is emitted immediately before that bank's recurrence
                # matmuls (its WAR dep -- last step's activation read -- is
                # ~a full step old, so it never stalls), keeping PE duty
                # high across the step boundary so HAM never re-throttles.
                # Bank order f, g, i, o; f/g lo-ci chunks wedge around the
                # deferred hi-transposes of the previous step's h.
                def xb(gp, c0, c1):
                    nc.tensor.matmul(
                        out=gp[:, 0 : c1 - c0], lhsT=sfull[:], rhs=xg_t[:, c0:c1],
                        start=True, stop=first, skip_group_check=True,
                    )

                if t > 0:
                    hT_hi_prev = stpool.tile(
                        [128, 2 * 128], bf16, tag="hThi", name=f"hThi_{t - 1}"
                    )

                    def rec_cis(gp, wcol0, ncol0, nlen, cis, ob_shift=0):
                        for ci in cis:
                            lhs = hT_lhs(hT_lo_prev, hT_hi_prev, ci)
                            for half in range(2):
                                base = wcol0 + 512 * half + ncol0 + ob_shift
                                nc.tensor.matmul(
                                    out=gp[64 * half : 64 * (half + 1), ncol0 : ncol0 + nlen],
                                    lhsT=lhs,
                                    rhs=wh_sb[ci][:, base : base + nlen],
                                    start=False,
                                    stop=(ci == 7),
                                    skip_group_check=True,
                                )

                    LO, HI = (0, 4, 1, 5), (2, 6, 3, 7)
                    xb(gp_f, 2 * HH, 3 * HH)
                    rec_cis(gp_f, 2 * HIDDEN, 0, HQ, LO)
                    rec_cis(gp_f, 2 * HIDDEN, HQ, HQ, LO)
                    # deferred hi-transposes of the previous step's h
                    for q in range(2):
                        nc.tensor.transpose(
                            out=tp_hi[:, 128 * q : 128 * (q + 1)],
                            in_=h_prev[2 + q][:],
                            identity=ident[:],
                        )
                    nc.vector.tensor_copy(hT_hi_prev[:], tp_hi[:, 0:256])
                    xb(gp_g, HH, 2 * HH)
                    rec_cis(gp_g, HIDDEN, 0, HQ, LO)
                    rec_cis(gp_f, 2 * HIDDEN, 0, HQ, HI)
                    rec_cis(gp_f, 2 * HIDDEN, HQ, HQ, HI)
                    rec_cis(gp_g, HIDDEN, 0, HQ, HI)
                    rec_cis(gp_g, HIDDEN, HQ, HQ, CI_ORDER)
                    xb(gp_ia, 0, HQ)
                    xb(gp_ib, HQ, HH)
                    rec_cis(gp_ia, 0, 0, HQ, CI_ORDER)
                    rec_cis(gp_ib, 0, 0, HQ, CI_ORDER, ob_shift=HQ)
                    xb(gp_oa, 3 * HH, 3 * HH + HQ)
                    rec_cis(gp_oa, 3 * HIDDEN, 0, HQ, CI_ORDER)
                    xb(gp_ob, 3 * HH + HQ, 4 * HH)
                    rec_cis(gp_ob, 3 * HIDDEN, 0, HQ, CI_ORDER, ob_shift=HQ)
                else:
                    x_pass(xg_t, stop=True)

                # ---- epilogue for step t (chunks a=[0:256], b=[256:512]) ----
                sl = [slice(0, HQ), slice(HQ, HH)]
                def at(nm, k, dt=f32, pool=None):
                    pool = pool or apool
                    return pool.tile([128, HQ], dt, tag=f"{nm}{k}", name=f"{nm}{k}_{t}")
                sigi = [at("sigi", k) for k in range(2)]
                tanhg = [at("tanhg", k) for k in range(2)]
                sigf = [at("sigf", k) for k in range(2)]
                tanhc = [at("tanhc", k) for k in range(2)]
                sigo = [at("sigo", k) for k in range(2)]
                m1 = [at("m1", k) for k in range(2)]
                cmul = [at("cmul", k) for k in range(2)]
                c_new = [at("c", k, pool=stpool) for k in range(2)]
                hq = [
                    apool.tile([128, 128], bf16, tag=f"hq{q}", name=f"hq{q}_{t}")
                    for q in range(4)
                ]

                # ACT queue (1.2 GHz): f, g, i chunk acts as banks complete,
                # then tanhc_a, sigo_a, tanhc_b, sigo_b
                for k in range(2):
                    nc.scalar.activation(
                        sigf[k][:], gp_f[:, sl[k]], AF.Sigmoid, bias=1.0
                    )
                for k in range(2):
                    nc.scalar.activation(tanhg[k][:], gp_g[:, sl[k]], AF.Tanh)
                gpi = [gp_ia, gp_ib]
                for k in range(2):
                    nc.scalar.activation(sigi[k][:], gpi[k][:, 0:HQ], AF.Sigmoid)
                # GPS queue (slow ALU, SBUF-only): cmul has slack mid-chain
                for k in range(2):
                    nc.gpsimd.tensor_mul(cmul[k][:], sigf[k][:], c_prev[k][:])
                # DVE queue: m1_a, cnew_a, m1_b, cnew_b (cnew_a asap)
                nc.vector.tensor_mul(m1[0][:], sigi[0][:], tanhg[0][:])
                nc.vector.tensor_add(c_new[0][:], cmul[0][:], m1[0][:])
                nc.vector.tensor_mul(m1[1][:], sigi[1][:], tanhg[1][:])
                nc.vector.tensor_add(c_new[1][:], cmul[1][:], m1[1][:])
                # ACT tail
                nc.scalar.activation(tanhc[0][:], c_new[0][:], AF.Tanh)
                nc.scalar.activation(sigo[0][:], gp_oa[:, 0:HQ], AF.Sigmoid)
                nc.scalar.activation(tanhc[1][:], c_new[1][:], AF.Tanh)
                nc.scalar.activation(sigo[1][:], gp_ob[:, 0:HQ], AF.Sigmoid)
                # DVE tail: h quarters q1/q2 feed this iteration's T0/T1
                for q in range(2):
                    k, c = divmod(128 * q, HQ)
                    nc.vector.tensor_mul(
                        hq[q][:], sigo[k][:, c : c + 128], tanhc[k][:, c : c + 128]
                    )

                # ---- h_a -> h^T low blocks (T0/T1) + cast ----
                hT_lo = stpool.tile([128, 2 * 128], bf16, tag="hTlo", name=f"hTlo_{t}")
                nc.tensor.transpose(
                    out=tp_lo[:, 0:128], in_=hq[0][:], identity=ident[:]
                )
                nc.tensor.transpose(
                    out=tp_lo[:, 128:256], in_=hq[1][:], identity=ident[:]
                )
                nc.vector.tensor_copy(hT_lo[:], tp_lo[:, 0:256])
                # h quarter q3 on gpsimd (feeds next iteration's T2p early,
                # off the DVE queue); q4 on DVE after cast_lo
                nc.gpsimd.tensor_mul(
                    hq[2][:], sigo[1][:, 0:128], tanhc[1][:, 0:128]
                )
                nc.vector.tensor_mul(
                    hq[3][:], sigo[1][:, 128:256], tanhc[1][:, 128:256]
                )

                c_prev = c_new
                hT_lo_prev = hT_lo
                h_prev = hq
                if not last:
                    xg_t = xg_n

            # ---- final step's hi transposes (deferred) ----
            hT_hi_prev = stpool.tile(
                [128, 2 * 128], bf16, tag="hThi", name="hThi_last"
            )
            for q in range(2):
                nc.tensor.transpose(
                    out=tp_hi[:, 128 * q : 128 * (q + 1)],
                    in_=h_prev[2 + q][:],
                    identity=ident[:],
                )
            nc.vector.tensor_copy(hT_hi_prev[:], tp_hi[:, 0:256])

            # ---- images encoder: out = images @ W_img + b_img ----
            # accumulator reuses gp_g's bank via the tag ring
            oip = pspool.tile([128, OUT // 2], f32, tag="gp_g", name="oip")
            nc.tensor.matmul(
                out=oip[:], lhsT=o2[:], rhs=bimg2[:],
                start=True, stop=False, skip_group_check=True,
            )
            for ci in range(16):
                lhs = imT_sl(ci)
                for half in range(2):
                    nc.tensor.matmul(
                        out=oip[64 * half : 64 * (half + 1), :],
                        lhsT=lhs,
                        rhs=wimg_sb[ci][:, 512 * half : 512 * (half + 1)],
                        start=False, stop=(ci == 15), skip_group_check=True,
                    )
            oimg_sb = opool.tile([128, OUT // 2], f32, tag="oimg")
            nc.vector.tensor_copy(oimg_sb[:], oip[:])
            nc.sync.dma_start(oimg_d[:], oimg_sb[:])

            # ---- hidden encoder: out = h @ W_hid + b_hid ----
            # reuse gp_f's bank (tag ring, bufs=1 -> same memory, WAR-tracked)
            ohp = pspool.tile([128, HH], f32, tag="gp_f", name="ohp")
            nc.tensor.matmul(
                out=ohp[:], lhsT=o2[:], rhs=bhid2[:],
                start=True, stop=False, skip_group_check=True,
            )
            for ci in range(8):
                lhs = hT_lhs(hT_lo_prev, hT_hi_prev, ci)
                for half in range(2):
                    nc.tensor.matmul(
                        out=ohp[64 * half : 64 * (half + 1), :],
                        lhsT=lhs,
                        rhs=whid_sb[ci][:, 512 * half : 512 * (half + 1)],
                        start=False, stop=(ci == 7), skip_group_check=True,
                    )
            ohid_sb = opool.tile([128, OUT // 2], f32, tag="ohid")
            nc.vector.tensor_copy(ohid_sb[:], ohp[:])
            nc.sync.dma_start(ohid_d[:], ohid_sb[:])

    nc.compile()
    return nc


def _host_prep(images, embed_table, W_cell, b_cell, W_img, b_img, W_hid, b_hid,
               message):
    """Builds the per-core input maps (all host-side preprocessing)."""
    from ml_dtypes import bfloat16

    W_x = W_cell[:EMB]          # [512, 4096]
    W_h = np.ascontiguousarray(W_cell[EMB:]).astype(bfloat16)  # [1024, 4096]

    M2 = embed_table.astype(np.float32) @ W_x + b_cell  # [1024, 4096]
    # (the f-gate +1.0 is applied as an activation bias on-device)
    # row 2v+h = [i_h, g_h, f_h, o_h] halves of vocab row v
    M2p = np.ascontiguousarray(
        M2.reshape(VOCAB, 4, 2, HH).transpose(0, 2, 1, 3).reshape(2 * VOCAB, 4 * HH)
    ).astype(bfloat16)

    sfull = np.zeros((2 * BS, 2 * BS), np.float32)
    for m in range(BS):
        sfull[2 * m, m] = 1.0
        sfull[2 * m + 1, BS + m] = 1.0
    sfull = sfull.astype(bfloat16)

    ident = np.eye(128, dtype=np.float32)

    o2 = np.zeros((2, 128), np.float32)
    o2[0, 0:64] = 1.0
    o2[1, 64:128] = 1.0

    W_img_b = W_img.astype(bfloat16)
    W_hid_b = W_hid.astype(bfloat16)
    bimg2 = np.stack([b_img[: OUT // 2], b_img[OUT // 2 :]]).astype(np.float32)
    bhid2 = np.stack([b_hid[: OUT // 2], b_hid[OUT // 2 :]]).astype(np.float32)

    in_maps = []
    for core in range(NCORES):
        slc = slice(core * BS, (core + 1) * BS)
        msg = message[slc]  # [64, T] int32
        msg2 = np.empty((2 * BS, T), np.int32)
        msg2[0::2] = 2 * msg
        msg2[1::2] = 2 * msg + 1
        in_maps.append(
            {
                "m2p": M2p,
                "wh": W_h,
                "msg2": msg2,
                "sfull": sfull,
                "ident": ident.astype(bfloat16),
                "identf": ident,
                "imgs": np.concatenate(
                    [images[slc, : D_IMG // 2], images[slc, D_IMG // 2 :]], axis=0
                ),
                "wimg": W_img_b,
                "whid": W_hid_b,
                "o2": o2,
                "bimg2": bimg2,
                "bhid2": bhid2,
            }
        )
    return in_maps


def kernel(images, embed_table, W_cell, b_cell, W_img, b_img, W_hid, b_hid,
           message):
    import sys
    if "/opt/trn_rl_repo" not in sys.path:
        sys.path.insert(0, "/opt/trn_rl_repo")
    from concourse.bass_utils import run_bass_kernel_spmd

    images = np.asarray(images, np.float32)
    embed_table = np.asarray(embed_table, np.float32)
    W_cell = np.asarray(W_cell, np.float32)
    b_cell = np.asarray(b_cell, np.float32)
    W_img = np.asarray(W_img, np.float32)
    b_img = np.asarray(b_img, np.float32)
    W_hid = np.asarray(W_hid, np.float32)
    b_hid = np.asarray(b_hid, np.float32)
    message = np.asarray(message, np.int32)

    n_steps = T
    if "nc" not in _CACHE or _CACHE.get("n_steps") != n_steps:
        _CACHE["nc"] = _build_nc(n_steps)
        _CACHE["n_steps"] = n_steps
    nc = _CACHE["nc"]

    in_maps = _host_prep(
        images, embed_table, W_cell, b_cell, W_img, b_img, W_hid, b_hid, message
    )
    res = run_bass_kernel_spmd(nc, in_maps, core_ids=list(range(NCORES)))
    results = res.results

    images_encoded = np.empty((B, OUT), np.float32)
    hidden_encoded = np.empty((B, OUT), np.float32)
    for core in range(NCORES):
        slc = slice(core * BS, (core + 1) * BS)
        oi = results[core]["oimg"]
        oh = results[core]["ohid"]
        images_encoded[slc, : OUT // 2] = oi[0:64]
        images_encoded[slc, OUT // 2 :] = oi[64:128]
        hidden_encoded[slc, : OUT // 2] = oh[0:64]
        hidden_encoded[slc, OUT // 2 :] = oh[64:128]
    return images_encoded, hidden_encoded


# revision 13
# speedup vs baseline: 1.4001x; 1.4001x over previous
"""Trainium2 Bass kernel for nn_Listener (LSTM listener + dense encoders).

Reference computation (per full batch B=512):
    emb = embed_table[message]                       # [B, T, 512]
    LSTM over T=128 steps, HIDDEN=1024:
        gated = [x_t, h] @ W_cell + b_cell           # [B, 4096] (i, g, f, o)
        f = sigmoid(f + 1); c = f*c + sigmoid(i)*tanh(g); h = sigmoid(o)*tanh(c)
    images_encoded = images @ W_img + b_img          # [B, 1024]
    hidden_encoded = h @ W_hid + b_hid               # [B, 1024]
    returns (images_encoded, hidden_encoded)

Strategy (8 NeuronCores, data-parallel over batch, 64 rows/core):
  * Embedding lookup + x-projection folded into one bf16 table:
        M2 = embed_table @ W_cell[:512] + b_cell   (gathered per token)
    injected into PSUM with a k=128 selection matmul that also seeds the
    accumulation groups (start=True).  The f-gate +1.0 is applied as an
    exact activation bias instead of being folded into the table.
  * Per-core batch 64 = half the PSUM partitions; hidden units split in
    half across partition ranges (partitions 0:64 <-> units u, 64:128 <->
    units u+512).  Recurrence matmul pairs (M=64) run concurrently in the
    two PE column groups -> full 128x128 bf16 utilization (216ns/pair).
  * The per-step critical path (gates -> c -> h -> h^T -> next matmul) is
    software-pipelined: every gate bank's matmuls are split into two
    256-column chunks so activations start as soon as each half-bank
    completes; every elementwise op is chunked with separate tiles
    (tile-granular deps); cmul/h run on the gpsimd engine; transposes are
    emitted per-h-chunk; next step's matmuls begin as soon as the low
    half of h^T is cast.  Bank order (f, g, i, o) gives the long
    c-chain (sigf->cmul->cnew->tanhc) a head start.
  * PSUM: 8 banks: gp_f, gp_g, gp_i, gp_oa, gp_ob, tp_lo, tp_hi, oip.
    The o-gate uses two half-banks so sigmoid(o) chunk a never collides
    with PE writes of chunk b.  The hidden-encoder accumulator reuses
    gp_f's bank via the tile-pool tag ring.
  * Encoder weights are DMA'd behind W_h during the recurrence; both
    encoders run on the PE right after the last step.
"""

import numpy as np

B, T = 512, 128
HIDDEN = 1024
VOCAB = 1024
EMB = 512
OUT = 1024
D_IMG = 2048
NCORES = 8
BS = B // NCORES  # 64 batch rows per core
HH = HIDDEN // 2  # 512 = per-half hidden units
HQ = HH // 2      # 256 = per-chunk columns

_CACHE = {}

# ci order: low blocks (h^T blocks 0,1) first, then high (blocks 2,3)
CI_ORDER = (0, 4, 1, 5, 2, 6, 3, 7)


def _build_nc(n_steps: int):
    import concourse.bass as bass
    import concourse.mybir as mybir
    from concourse import bacc, tile

    f32 = mybir.dt.float32
    f32r = mybir.dt.float32r
    bf16 = mybir.dt.bfloat16
    i32 = mybir.dt.int32
    AF = mybir.ActivationFunctionType

    nc = bacc.Bacc("TRN2", target_bir_lowering=False, debug=False)

    m2p_d = nc.declare_dram_parameter("m2p", [2 * VOCAB, HH * 4], bf16, isOutput=False)
    wh_d = nc.declare_dram_parameter("wh", [HIDDEN, 4 * HIDDEN], bf16, isOutput=False)
    msg2_d = nc.declare_dram_parameter("msg2", [2 * BS, T], i32, isOutput=False)
    sfull_d = nc.declare_dram_parameter("sfull", [2 * BS, 2 * BS], bf16, isOutput=False)
    ident_d = nc.declare_dram_parameter("ident", [128, 128], bf16, isOutput=False)
    identf_d = nc.declare_dram_parameter("identf", [128, 128], f32, isOutput=False)
    imgs_d = nc.declare_dram_parameter("imgs", [128, D_IMG // 2], f32, isOutput=False)
    wimg_d = nc.declare_dram_parameter("wimg", [D_IMG, OUT], bf16, isOutput=False)
    whid_d = nc.declare_dram_parameter("whid", [HIDDEN, OUT], bf16, isOutput=False)
    o2_d = nc.declare_dram_parameter("o2", [2, 128], f32r, isOutput=False)
    bimg2_d = nc.declare_dram_parameter("bimg2", [2, OUT // 2], f32r, isOutput=False)
    bhid2_d = nc.declare_dram_parameter("bhid2", [2, OUT // 2], f32r, isOutput=False)
    oimg_d = nc.declare_dram_parameter("oimg", [128, OUT // 2], f32, isOutput=True)
    ohid_d = nc.declare_dram_parameter("ohid", [128, OUT // 2], f32, isOutput=True)

    with tile.TileContext(nc) as tc:
        with (
            tc.tile_pool(name="wpool", bufs=1) as wpool,
            tc.tile_pool(name="const", bufs=1) as cpool,
            tc.tile_pool(name="xg", bufs=2) as xgpool,
            tc.tile_pool(name="state", bufs=2) as stpool,
            tc.tile_pool(name="act", bufs=1) as apool,
            tc.tile_pool(name="outs", bufs=1) as opool,
            tc.tile_pool(name="psum", bufs=1, space="PSUM") as pspool,
        ):
            # ---- constants / small inputs ----
            msg2 = cpool.tile([2 * BS, T], i32, tag="msg2")
            nc.sync.dma_start(msg2[:], msg2_d[:])
            sfull = cpool.tile([2 * BS, 2 * BS], bf16, tag="sfull")
            nc.sync.dma_start(sfull[:], sfull_d[:])
            ident = cpool.tile([128, 128], bf16, tag="ident")
            nc.sync.dma_start(ident[:], ident_d[:])
            identf = cpool.tile([128, 128], f32, tag="identf")
            nc.sync.dma_start(identf[:], identf_d[:])
            o2 = cpool.tile([2, 128], f32r, tag="o2")
            nc.sync.dma_start(o2[:], o2_d[:])
            bimg2 = cpool.tile([2, OUT // 2], f32r, tag="bimg2")
            nc.sync.dma_start(bimg2[:], bimg2_d[:])
            bhid2 = cpool.tile([2, OUT // 2], f32r, tag="bhid2")
            nc.sync.dma_start(bhid2[:], bhid2_d[:])
            imgs = cpool.tile([128, D_IMG // 2], f32, tag="imgs")
            nc.sync.dma_start(imgs[:], imgs_d[:])

            # ---- weights: W_h first (step-1 critical), then W_img, W_hid ----
            wh_sb = []
            for ci in range(8):
                wt = wpool.tile([128, 4 * HIDDEN], bf16, tag=f"wh{ci}")
                nc.sync.dma_start(wt[:], wh_d[128 * ci : 128 * (ci + 1), :])
                wh_sb.append(wt)
            wimg_sb = []
            for ci in range(16):
                wt = wpool.tile([128, OUT], bf16, tag=f"wimg{ci}")
                nc.sync.dma_start(wt[:], wimg_d[128 * ci : 128 * (ci + 1), :])
                wimg_sb.append(wt)
            whid_sb = []
            for ci in range(8):
                wt = wpool.tile([128, OUT], bf16, tag=f"whid{ci}")
                nc.sync.dma_start(wt[:], whid_d[128 * ci : 128 * (ci + 1), :])
                whid_sb.append(wt)

            # ---- PSUM layout: 8 banks, one full-bank tile per bank.
            # The i and o gates each get TWO banks (a/b column chunks in
            # separate banks) so their chunk activations can start while the
            # PE still writes the other chunk (engine-read + PE-write of the
            # same bank is fatal; tile deps are tile-granular).  The encoder
            # accumulators reuse gate banks after the loop via the tag ring.
            gp_f = pspool.tile([128, HH], f32, tag="gp_f")    # bank 0
            gp_g = pspool.tile([128, HH], f32, tag="gp_g")    # bank 1
            gp_ia = pspool.tile([128, HH], f32, tag="gp_ia")  # bank 2 (cols 0:256)
            gp_ib = pspool.tile([128, HH], f32, tag="gp_ib")  # bank 3 (cols 0:256)
            gp_oa = pspool.tile([128, HH], f32, tag="gp_oa")  # bank 4 (cols 0:256)
            gp_ob = pspool.tile([128, HH], f32, tag="gp_ob")  # bank 5 (cols 0:256)
            tp_lo = pspool.tile([128, 2 * HH], bf16, tag="tp_lo")  # bank 6
            tp_hi = pspool.tile([128, 2 * HH], bf16, tag="tp_hi")  # bank 7

            # ---- images transposed up front (also PE warmup) ----
            # imgs packed [128, 1024]: partitions 0:64 = batch x feats 0:1024,
            # 64:128 = batch x feats 1024:2048.
            imT = cpool.tile([128, D_IMG // 2], bf16, tag="imT")
            for half in range(2):
                tph = pspool.tile(
                    [128, HH], f32, tag=("gp_f", "gp_g")[half], name=f"tpim{half}"
                )
                for q in range(4):
                    qq = 4 * half + q
                    nc.tensor.transpose(
                        out=tph[:, 128 * q : 128 * (q + 1)],
                        in_=imgs[:, 128 * qq : 128 * (qq + 1)],
                        identity=identf[:],
                    )
                nc.vector.tensor_copy(imT[:, 512 * half : 512 * (half + 1)], tph[:])

            def imT_sl(ci):
                q, hi = (ci - 8, 64) if ci >= 8 else (ci, 0)
                return imT[:, 128 * q + hi : 128 * q + hi + 64]

            # ---- LSTM state init (c in two chunk tiles, double-buffered) ----
            c_prev = []
            for k in range(2):
                ct = stpool.tile([128, HQ], f32, tag=f"c{k}")
                nc.vector.memset(ct[:], 0.0)
                c_prev.append(ct)

            def gather(t):
                xg = xgpool.tile([2 * BS, 4 * HH], bf16, tag="xg", name=f"xg_{t}")
                nc.gpsimd.indirect_dma_start(
                    out=xg[:], out_offset=None, in_=m2p_d[:],
                    in_offset=bass.IndirectOffsetOnAxis(ap=msg2[:, t : t + 1], axis=0),
                )
                return xg

            def x_pass(xg, stop):
                # PSUM gate banks seeded with the gathered x-contribution
                for gp, c0, c1 in (
                    (gp_f, 2 * HH, 3 * HH),
                    (gp_g, HH, 2 * HH),
                    (gp_ia, 0, HQ),
                    (gp_ib, HQ, HH),
                    (gp_oa, 3 * HH, 3 * HH + HQ),
                    (gp_ob, 3 * HH + HQ, 4 * HH),
                ):
                    nc.tensor.matmul(
                        out=gp[:, 0 : c1 - c0], lhsT=sfull[:], rhs=xg[:, c0:c1],
                        start=True, stop=stop, skip_group_check=True,
                    )

            def hT_lhs(hT_lo, hT_hi, ci):
                q, hi = (ci - 4, 64) if ci >= 4 else (ci, 0)
                src = hT_lo if q < 2 else hT_hi
                return src[:, 128 * (q % 2) + hi : 128 * (q % 2) + hi + 64]

            xg_t = gather(0)

            hT_lo_prev = hT_hi_prev = h_prev = None

            for t in range(n_steps):
                last = t == n_steps - 1
                first = t == 0
                # gather for step t+1 (gpsimd queue head; xg double-buffered)
                if not last:
                    xg_n = gather(t + 1)

                # ---- PE stream for step t: the X seed matmul of each gate
                # bank is emitted immediately before that bank's recurrence
                # matmuls (its WAR dep -- last step's activation read -- is
                # ~a full step old, so it never stalls), keeping PE duty
                # high across the step boundary so HAM never re-throttles.
                # Bank order f, g, i, o; f/g lo-ci chunks wedge around the
                # deferred hi-transposes of the previous step's h.
                def xb(gp, c0, c1):
                    nc.tensor.matmul(
                        out=gp[:, 0 : c1 - c0], lhsT=sfull[:], rhs=xg_t[:, c0:c1],
                        start=True, stop=first, skip_group_check=True,
                    )

                if t > 0:
                    hT_hi_prev = stpool.tile(
                        [128, 2 * 128], bf16, tag="hThi", name=f"hThi_{t - 1}"
                    )

                    def rec_cis(gp, wcol0, ncol0, nlen, cis, ob_shift=0):
                        for ci in cis:
                            lhs = hT_lhs(hT_lo_prev, hT_hi_prev, ci)
                            for half in range(2):
                                base = wcol0 + 512 * half + ncol0 + ob_shift
                                nc.tensor.matmul(
                                    out=gp[64 * half : 64 * (half + 1), ncol0 : ncol0 + nlen],
                                    lhsT=lhs,
                                    rhs=wh_sb[ci][:, base : base + nlen],
                                    start=False,
                                    stop=(ci == 7),
                                    skip_group_check=True,
                                )

                    LO, HI = (0, 4, 1, 5), (2, 6, 3, 7)
                    xb(gp_f, 2 * HH, 3 * HH)
                    rec_cis(gp_f, 2 * HIDDEN, 0, HQ, LO)
                    rec_cis(gp_f, 2 * HIDDEN, HQ, HQ, LO)
                    # deferred hi-transposes of the previous step's h
                    for q in range(2):
                        nc.tensor.transpose(
                            out=tp_hi[:, 128 * q : 128 * (q + 1)],
                            in_=h_prev[2 + q][:],
                            identity=ident[:],
                        )
                    nc.vector.tensor_copy(hT_hi_prev[:], tp_hi[:, 0:256])
                    xb(gp_g, HH, 2 * HH)
                    rec_cis(gp_g, HIDDEN, 0, HQ, LO)
                    rec_cis(gp_f, 2 * HIDDEN, 0, HQ, HI)
                    rec_cis(gp_f, 2 * HIDDEN, HQ, HQ, HI)
                    rec_cis(gp_g, HIDDEN, 0, HQ, HI)
                    rec_cis(gp_g, HIDDEN, HQ, HQ, CI_ORDER)
                    xb(gp_ia, 0, HQ)
                    xb(gp_ib, HQ, HH)
                    rec_cis(gp_ia, 0, 0, HQ, CI_ORDER)
                    rec_cis(gp_ib, 0, 0, HQ, CI_ORDER, ob_shift=HQ)
                    xb(gp_oa, 3 * HH, 3 * HH + HQ)
                    rec_cis(gp_oa, 3 * HIDDEN, 0, HQ, CI_ORDER)
                    xb(gp_ob, 3 * HH + HQ, 4 * HH)
                    rec_cis(gp_ob, 3 * HIDDEN, 0, HQ, CI_ORDER, ob_shift=HQ)
                else:
                    x_pass(xg_t, stop=True)

                # ---- epilogue for step t (chunks a=[0:256], b=[256:512]) ----
                sl = [slice(0, HQ), slice(HQ, HH)]
                def at(nm, k, dt=f32, pool=None):
                    pool = pool or apool
                    return pool.tile([128, HQ], dt, tag=f"{nm}{k}", name=f"{nm}{k}_{t}")
                sigi = [at("sigi", k) for k in range(2)]
                tanhg = [at("tanhg", k) for k in range(2)]
                sigf = [at("sigf", k) for k in range(2)]
                tanhc = [at("tanhc", k) for k in range(2)]
                sigo = [at("sigo", k) for k in range(2)]
                m1 = [at("m1", k) for k in range(2)]
                cmul = [at("cmul", k) for k in range(2)]
                c_new = [at("c", k, pool=stpool) for k in range(2)]
                hq = [
                    apool.tile([128, 128], bf16, tag=f"hq{q}", name=f"hq{q}_{t}")
                    for q in range(4)
                ]

                # ACT queue (1.2 GHz): f, g, i chunk acts as banks complete,
                # then tanhc_a, sigo_a, tanhc_b, sigo_b
                for k in range(2):
                    nc.scalar.activation(
                        sigf[k][:], gp_f[:, sl[k]], AF.Sigmoid, bias=1.0
                    )
                for k in range(2):
                    nc.scalar.activation(tanhg[k][:], gp_g[:, sl[k]], AF.Tanh)
                gpi = [gp_ia, gp_ib]
                for k in range(2):
                    nc.scalar.activation(sigi[k][:], gpi[k][:, 0:HQ], AF.Sigmoid)
                # GPS queue (slow ALU, SBUF-only): cmul has slack mid-chain
                for k in range(2):
                    nc.gpsimd.tensor_mul(cmul[k][:], sigf[k][:], c_prev[k][:])
                # DVE queue: m1_a, cnew_a, m1_b, cnew_b (cnew_a asap)
                nc.vector.tensor_mul(m1[0][:], sigi[0][:], tanhg[0][:])
                nc.vector.tensor_add(c_new[0][:], cmul[0][:], m1[0][:])
                nc.vector.tensor_mul(m1[1][:], sigi[1][:], tanhg[1][:])
                nc.vector.tensor_add(c_new[1][:], cmul[1][:], m1[1][:])
                # ACT tail.  tanhc_b carries an artificial zero-bias dep on
                # sigo_a: the scheduler's CoreSim cost model runs the PE ~2x
                # slower than reality (no column-tile concurrency), so without
                # the dep it enqueues tanhc_b before sigo_a in the strict ACT
                # FIFO and sigo_a (+the whole h/transpose/cast tail) blocks
                # ~1.5us behind tanhc_b's slow c-chain inputs.
                zb = apool.tile([128, 1], f32, tag="zb", name=f"zb_{t}")
                nc.scalar.activation(tanhc[0][:], c_new[0][:], AF.Tanh)
                nc.scalar.activation(sigo[0][:], gp_oa[:, 0:HQ], AF.Sigmoid)
                nc.vector.tensor_scalar_mul(zb[:], sigo[0][:, 0:1], 0.0)
                nc.scalar.activation(tanhc[1][:], c_new[1][:], AF.Tanh, bias=zb[:])
                nc.scalar.activation(sigo[1][:], gp_ob[:, 0:HQ], AF.Sigmoid)
                # DVE tail: h quarters q1/q2 feed this iteration's T0/T1
                for q in range(2):
                    k, c = divmod(128 * q, HQ)
                    nc.vector.tensor_mul(
                        hq[q][:], sigo[k][:, c : c + 128], tanhc[k][:, c : c + 128]
                    )

                # ---- h_a -> h^T low blocks (T0/T1) + cast ----
                hT_lo = stpool.tile([128, 2 * 128], bf16, tag="hTlo", name=f"hTlo_{t}")
                nc.tensor.transpose(
                    out=tp_lo[:, 0:128], in_=hq[0][:], identity=ident[:]
                )
                nc.tensor.transpose(
                    out=tp_lo[:, 128:256], in_=hq[1][:], identity=ident[:]
                )
                nc.vector.tensor_copy(hT_lo[:], tp_lo[:, 0:256])
                # h quarters q3/q4 on gpsimd (their transposes run early in
                # the next iteration; keeping them off the DVE queue keeps the
                # scheduler from displacing cast_lo)
                nc.gpsimd.tensor_mul(
                    hq[2][:], sigo[1][:, 0:128], tanhc[1][:, 0:128]
                )
                nc.gpsimd.tensor_mul(
                    hq[3][:], sigo[1][:, 128:256], tanhc[1][:, 128:256]
                )

                c_prev = c_new
                hT_lo_prev = hT_lo
                h_prev = hq
                if not last:
                    xg_t = xg_n

            # ---- final step's hi transposes (deferred) ----
            hT_hi_prev = stpool.tile(
                [128, 2 * 128], bf16, tag="hThi", name="hThi_last"
            )
            for q in range(2):
                nc.tensor.transpose(
                    out=tp_hi[:, 128 * q : 128 * (q + 1)],
                    in_=h_prev[2 + q][:],
                    identity=ident[:],
                )
            nc.vector.tensor_copy(hT_hi_prev[:], tp_hi[:, 0:256])

            # ---- images encoder: out = images @ W_img + b_img ----
            # accumulator reuses gp_g's bank via the tag ring
            oip = pspool.tile([128, OUT // 2], f32, tag="gp_g", name="oip")
            nc.tensor.matmul(
                out=oip[:], lhsT=o2[:], rhs=bimg2[:],
                start=True, stop=False, skip_group_check=True,
            )
            for ci in range(16):
                lhs = imT_sl(ci)
                for half in range(2):
                    nc.tensor.matmul(
                        out=oip[64 * half : 64 * (half + 1), :],
                        lhsT=lhs,
                        rhs=wimg_sb[ci][:, 512 * half : 512 * (half + 1)],
                        start=False, stop=(ci == 15), skip_group_check=True,
                    )
            oimg_sb = opool.tile([128, OUT // 2], f32, tag="oimg")
            nc.vector.tensor_copy(oimg_sb[:], oip[:])
            nc.sync.dma_start(oimg_d[:], oimg_sb[:])

            # ---- hidden encoder: out = h @ W_hid + b_hid ----
            # reuse gp_f's bank (tag ring, bufs=1 -> same memory, WAR-tracked)
            ohp = pspool.tile([128, HH], f32, tag="gp_f", name="ohp")
            nc.tensor.matmul(
                out=ohp[:], lhsT=o2[:], rhs=bhid2[:],
                start=True, stop=False, skip_group_check=True,
            )
            for ci in range(8):
                lhs = hT_lhs(hT_lo_prev, hT_hi_prev, ci)
                for half in range(2):
                    nc.tensor.matmul(
                        out=ohp[64 * half : 64 * (half + 1), :],
                        lhsT=lhs,
                        rhs=whid_sb[ci][:, 512 * half : 512 * (half + 1)],
                        start=False, stop=(ci == 7), skip_group_check=True,
                    )
            ohid_sb = opool.tile([128, OUT // 2], f32, tag="ohid")
            nc.vector.tensor_copy(ohid_sb[:], ohp[:])
            nc.sync.dma_start(ohid_d[:], ohid_sb[:])

    nc.compile()
    return nc


def _host_prep(images, embed_table, W_cell, b_cell, W_img, b_img, W_hid, b_hid,
               message):
    """Builds the per-core input maps (all host-side preprocessing)."""
    from ml_dtypes import bfloat16

    W_x = W_cell[:EMB]          # [512, 4096]
    W_h = np.ascontiguousarray(W_cell[EMB:]).astype(bfloat16)  # [1024, 4096]

    M2 = embed_table.astype(np.float32) @ W_x + b_cell  # [1024, 4096]
    # (the f-gate +1.0 is applied as an activation bias on-device)
    # row 2v+h = [i_h, g_h, f_h, o_h] halves of vocab row v
    M2p = np.ascontiguousarray(
        M2.reshape(VOCAB, 4, 2, HH).transpose(0, 2, 1, 3).reshape(2 * VOCAB, 4 * HH)
    ).astype(bfloat16)

    sfull = np.zeros((2 * BS, 2 * BS), np.float32)
    for m in range(BS):
        sfull[2 * m, m] = 1.0
        sfull[2 * m + 1, BS + m] = 1.0
    sfull = sfull.astype(bfloat16)

    ident = np.eye(128, dtype=np.float32)

    o2 = np.zeros((2, 128), np.float32)
    o2[0, 0:64] = 1.0
    o2[1, 64:128] = 1.0

    W_img_b = W_img.astype(bfloat16)
    W_hid_b = W_hid.astype(bfloat16)
    bimg2 = np.stack([b_img[: OUT // 2], b_img[OUT // 2 :]]).astype(np.float32)
    bhid2 = np.stack([b_hid[: OUT // 2], b_hid[OUT // 2 :]]).astype(np.float32)

    in_maps = []
    for core in range(NCORES):
        slc = slice(core * BS, (core + 1) * BS)
        msg = message[slc]  # [64, T] int32
        msg2 = np.empty((2 * BS, T), np.int32)
        msg2[0::2] = 2 * msg
        msg2[1::2] = 2 * msg + 1
        in_maps.append(
            {
                "m2p": M2p,
                "wh": W_h,
                "msg2": msg2,
                "sfull": sfull,
                "ident": ident.astype(bfloat16),
                "identf": ident,
                "imgs": np.concatenate(
                    [images[slc, : D_IMG // 2], images[slc, D_IMG // 2 :]], axis=0
                ),
                "wimg": W_img_b,
                "whid": W_hid_b,
                "o2": o2,
                "bimg2": bimg2,
                "bhid2": bhid2,
            }
        )
    return in_maps


def kernel(images, embed_table, W_cell, b_cell, W_img, b_img, W_hid, b_hid,
           message):
    import sys
    if "/opt/trn_rl_repo" not in sys.path:
        sys.path.insert(0, "/opt/trn_rl_repo")
    from concourse.bass_utils import run_bass_kernel_spmd

    images = np.asarray(images, np.float32)
    embed_table = np.asarray(embed_table, np.float32)
    W_cell = np.asarray(W_cell, np.float32)
    b_cell = np.asarray(b_cell, np.float32)
    W_img = np.asarray(W_img, np.float32)
    b_img = np.asarray(b_img, np.float32)
    W_hid = np.asarray(W_hid, np.float32)
    b_hid = np.asarray(b_hid, np.float32)
    message = np.asarray(message, np.int32)

    n_steps = T
    if "nc" not in _CACHE or _CACHE.get("n_steps") != n_steps:
        _CACHE["nc"] = _build_nc(n_steps)
        _CACHE["n_steps"] = n_steps
    nc = _CACHE["nc"]

    in_maps = _host_prep(
        images, embed_table, W_cell, b_cell, W_img, b_img, W_hid, b_hid, message
    )
    res = run_bass_kernel_spmd(nc, in_maps, core_ids=list(range(NCORES)))
    results = res.results

    images_encoded = np.empty((B, OUT), np.float32)
    hidden_encoded = np.empty((B, OUT), np.float32)
    for core in range(NCORES):
        slc = slice(core * BS, (core + 1) * BS)
        oi = results[core]["oimg"]
        oh = results[core]["ohid"]
        images_encoded[slc, : OUT // 2] = oi[0:64]
        images_encoded[slc, OUT // 2 :] = oi[64:128]
        hidden_encoded[slc, : OUT // 2] = oh[0:64]
        hidden_encoded[slc, OUT // 2 :] = oh[64:128]
    return images_encoded, hidden_encoded


# revision 16
# speedup vs baseline: 1.4068x; 1.0048x over previous
"""Trainium2 Bass kernel for nn_Listener (LSTM listener + dense encoders).

Reference computation (per full batch B=512):
    emb = embed_table[message]                       # [B, T, 512]
    LSTM over T=128 steps, HIDDEN=1024:
        gated = [x_t, h] @ W_cell + b_cell           # [B, 4096] (i, g, f, o)
        f = sigmoid(f + 1); c = f*c + sigmoid(i)*tanh(g); h = sigmoid(o)*tanh(c)
    images_encoded = images @ W_img + b_img          # [B, 1024]
    hidden_encoded = h @ W_hid + b_hid               # [B, 1024]
    returns (images_encoded, hidden_encoded)

Strategy (8 NeuronCores, data-parallel over batch, 64 rows/core):
  * Embedding lookup + x-projection folded into one bf16 table:
        M2 = embed_table @ W_cell[:512] + b_cell   (gathered per token)
    injected into PSUM with a k=128 selection matmul that also seeds the
    accumulation groups (start=True).  The f-gate +1.0 is applied as an
    exact activation bias instead of being folded into the table.
  * Per-core batch 64 = half the PSUM partitions; hidden units split in
    half across partition ranges (partitions 0:64 <-> units u, 64:128 <->
    units u+512).  Recurrence matmul pairs (M=64) run concurrently in the
    two PE column groups -> full 128x128 bf16 utilization (216ns/pair).
  * The per-step critical path (gates -> c -> h -> h^T -> next matmul) is
    software-pipelined: every gate bank's matmuls are split into two
    256-column chunks so activations start as soon as each half-bank
    completes; every elementwise op is chunked with separate tiles
    (tile-granular deps); cmul/h run on the gpsimd engine; transposes are
    emitted per-h-chunk; next step's matmuls begin as soon as the low
    half of h^T is cast.  Bank order (f, g, i, o) gives the long
    c-chain (sigf->cmul->cnew->tanhc) a head start.
  * PSUM: 8 banks: gp_f, gp_g, gp_i, gp_oa, gp_ob, tp_lo, tp_hi, oip.
    The o-gate uses two half-banks so sigmoid(o) chunk a never collides
    with PE writes of chunk b.  The hidden-encoder accumulator reuses
    gp_f's bank via the tile-pool tag ring.
  * Encoder weights are DMA'd behind W_h during the recurrence; both
    encoders run on the PE right after the last step.
"""

import numpy as np

B, T = 512, 128
HIDDEN = 1024
VOCAB = 1024
EMB = 512
OUT = 1024
D_IMG = 2048
NCORES = 8
BS = B // NCORES  # 64 batch rows per core
HH = HIDDEN // 2  # 512 = per-half hidden units
HQ = HH // 2      # 256 = per-chunk columns

_CACHE = {}

# ci order: low blocks (h^T blocks 0,1) first, then high (blocks 2,3)
CI_ORDER = (0, 4, 1, 5, 2, 6, 3, 7)


def _build_nc(n_steps: int):
    import concourse.bass as bass
    import concourse.mybir as mybir
    from concourse import bacc, tile

    f32 = mybir.dt.float32
    f32r = mybir.dt.float32r
    bf16 = mybir.dt.bfloat16
    i32 = mybir.dt.int32
    AF = mybir.ActivationFunctionType

    nc = bacc.Bacc("TRN2", target_bir_lowering=False, debug=False)

    m2p_d = nc.declare_dram_parameter("m2p", [2 * VOCAB, HH * 4], bf16, isOutput=False)
    wh_d = nc.declare_dram_parameter("wh", [HIDDEN, 4 * HIDDEN], bf16, isOutput=False)
    msg2_d = nc.declare_dram_parameter("msg2", [2 * BS, T], i32, isOutput=False)
    sfull_d = nc.declare_dram_parameter("sfull", [2 * BS, 2 * BS], bf16, isOutput=False)
    ident_d = nc.declare_dram_parameter("ident", [128, 128], bf16, isOutput=False)
    identf_d = nc.declare_dram_parameter("identf", [128, 128], f32, isOutput=False)
    imgs_d = nc.declare_dram_parameter("imgs", [128, D_IMG // 2], f32, isOutput=False)
    wimg_d = nc.declare_dram_parameter("wimg", [D_IMG, OUT], bf16, isOutput=False)
    whid_d = nc.declare_dram_parameter("whid", [HIDDEN, OUT], bf16, isOutput=False)
    o2_d = nc.declare_dram_parameter("o2", [2, 128], f32r, isOutput=False)
    bimg2_d = nc.declare_dram_parameter("bimg2", [2, OUT // 2], f32r, isOutput=False)
    bhid2_d = nc.declare_dram_parameter("bhid2", [2, OUT // 2], f32r, isOutput=False)
    oimg_d = nc.declare_dram_parameter("oimg", [128, OUT // 2], f32, isOutput=True)
    ohid_d = nc.declare_dram_parameter("ohid", [128, OUT // 2], f32, isOutput=True)

    with tile.TileContext(nc) as tc:
        with (
            tc.tile_pool(name="wpool", bufs=1) as wpool,
            tc.tile_pool(name="const", bufs=1) as cpool,
            tc.tile_pool(name="xg", bufs=2) as xgpool,
            tc.tile_pool(name="state", bufs=2) as stpool,
            tc.tile_pool(name="act", bufs=1) as apool,
            tc.tile_pool(name="outs", bufs=1) as opool,
            tc.tile_pool(name="psum", bufs=1, space="PSUM") as pspool,
        ):
            # ---- constants / small inputs ----
            msg2 = cpool.tile([2 * BS, T], i32, tag="msg2")
            nc.sync.dma_start(msg2[:], msg2_d[:])
            sfull = cpool.tile([2 * BS, 2 * BS], bf16, tag="sfull")
            nc.sync.dma_start(sfull[:], sfull_d[:])
            ident = cpool.tile([128, 128], bf16, tag="ident")
            nc.sync.dma_start(ident[:], ident_d[:])
            identf = cpool.tile([128, 128], f32, tag="identf")
            nc.sync.dma_start(identf[:], identf_d[:])
            o2 = cpool.tile([2, 128], f32r, tag="o2")
            nc.sync.dma_start(o2[:], o2_d[:])
            bimg2 = cpool.tile([2, OUT // 2], f32r, tag="bimg2")
            nc.sync.dma_start(bimg2[:], bimg2_d[:])
            bhid2 = cpool.tile([2, OUT // 2], f32r, tag="bhid2")
            nc.sync.dma_start(bhid2[:], bhid2_d[:])
            imgs = cpool.tile([128, D_IMG // 2], f32, tag="imgs")
            nc.sync.dma_start(imgs[:], imgs_d[:])

            # ---- weights: W_h first (step-1 critical), then W_img, W_hid ----
            wh_sb = []
            for ci in range(8):
                wt = wpool.tile([128, 4 * HIDDEN], bf16, tag=f"wh{ci}")
                nc.sync.dma_start(wt[:], wh_d[128 * ci : 128 * (ci + 1), :])
                wh_sb.append(wt)
            wimg_sb = []
            for ci in range(16):
                wt = wpool.tile([128, OUT], bf16, tag=f"wimg{ci}")
                nc.sync.dma_start(wt[:], wimg_d[128 * ci : 128 * (ci + 1), :])
                wimg_sb.append(wt)
            whid_sb = []
            for ci in range(8):
                wt = wpool.tile([128, OUT], bf16, tag=f"whid{ci}")
                nc.sync.dma_start(wt[:], whid_d[128 * ci : 128 * (ci + 1), :])
                whid_sb.append(wt)

            # ---- PSUM layout: 8 banks, one full-bank tile per bank.
            # The i and o gates each get TWO banks (a/b column chunks in
            # separate banks) so their chunk activations can start while the
            # PE still writes the other chunk (engine-read + PE-write of the
            # same bank is fatal; tile deps are tile-granular).  The encoder
            # accumulators reuse gate banks after the loop via the tag ring.
            gp_f = pspool.tile([128, HH], f32, tag="gp_f")    # bank 0
            gp_g = pspool.tile([128, HH], f32, tag="gp_g")    # bank 1
            gp_ia = pspool.tile([128, HH], f32, tag="gp_ia")  # bank 2 (cols 0:256)
            gp_ib = pspool.tile([128, HH], f32, tag="gp_ib")  # bank 3 (cols 0:256)
            gp_oa = pspool.tile([128, HH], f32, tag="gp_oa")  # bank 4 (cols 0:256)
            gp_ob = pspool.tile([128, HH], f32, tag="gp_ob")  # bank 5 (cols 0:256)
            tp_lo = pspool.tile([128, 2 * HH], bf16, tag="tp_lo")  # bank 6
            tp_hi = pspool.tile([128, 2 * HH], bf16, tag="tp_hi")  # bank 7

            # ---- images transposed up front (also PE warmup) ----
            # imgs packed [128, 1024]: partitions 0:64 = batch x feats 0:1024,
            # 64:128 = batch x feats 1024:2048.
            imT = cpool.tile([128, D_IMG // 2], bf16, tag="imT")
            for half in range(2):
                tph = pspool.tile(
                    [128, HH], f32, tag=("gp_f", "gp_g")[half], name=f"tpim{half}"
                )
                for q in range(4):
                    qq = 4 * half + q
                    nc.tensor.transpose(
                        out=tph[:, 128 * q : 128 * (q + 1)],
                        in_=imgs[:, 128 * qq : 128 * (qq + 1)],
                        identity=identf[:],
                    )
                nc.vector.tensor_copy(imT[:, 512 * half : 512 * (half + 1)], tph[:])

            def imT_sl(ci):
                q, hi = (ci - 8, 64) if ci >= 8 else (ci, 0)
                return imT[:, 128 * q + hi : 128 * q + hi + 64]

            # ---- LSTM state init (c in two chunk tiles, double-buffered) ----
            c_prev = []
            for k in range(2):
                ct = stpool.tile([128, HQ], f32, tag=f"c{k}")
                nc.vector.memset(ct[:], 0.0)
                c_prev.append(ct)

            def gather(t):
                xg = xgpool.tile([2 * BS, 4 * HH], bf16, tag="xg", name=f"xg_{t}")
                nc.gpsimd.indirect_dma_start(
                    out=xg[:], out_offset=None, in_=m2p_d[:],
                    in_offset=bass.IndirectOffsetOnAxis(ap=msg2[:, t : t + 1], axis=0),
                )
                return xg

            def hT_lhs(hT_lo, hT_hi, ci):
                q, hi = (ci - 4, 64) if ci >= 4 else (ci, 0)
                src = hT_lo if q < 2 else hT_hi
                return src[:, 128 * (q % 2) + hi : 128 * (q % 2) + hi + 64]

            xg_t = gather(0)

            hT_lo_prev = hT_hi_prev = h_prev = None

            for t in range(n_steps):
                last = t == n_steps - 1
                first = t == 0
                # gather for step t+1 (gpsimd queue head; xg double-buffered)
                if not last:
                    xg_n = gather(t + 1)

                # ---- PE stream for step t: the X seed matmul of each gate
                # bank is emitted immediately before that bank's recurrence
                # matmuls (its WAR dep -- last step's activation read -- is
                # ~a full step old, so it never stalls), keeping PE duty
                # high across the step boundary so HAM never re-throttles.
                # Bank order f, g, i, o; f/g lo-ci chunks wedge around the
                # deferred hi-transposes of the previous step's h.
                def xb(gp, c0, c1):
                    # col-tiled M=64 pairs (same shape as the rec pairs, so
                    # they pipeline at full rate in the PE stream); both
                    # halves stream the same xg columns -- the gather's
                    # 2-rows-per-batch packing already encodes the half split.
                    # start=True clears has_written for the whole bank WITHIN
                    # the partitions the matmul writes: each half's first
                    # column chunk carries start=True, later chunks of the
                    # same half overwrite-where-clear with start=False.
                    for nc0 in range(0, c1 - c0, HQ):
                        for half in range(2):
                            nc.tensor.matmul(
                                out=gp[64 * half : 64 * (half + 1), nc0 : nc0 + HQ],
                                lhsT=sfull[:, 64 * half : 64 * (half + 1)],
                                rhs=xg_t[:, c0 + nc0 : c0 + nc0 + HQ],
                                start=(nc0 == 0), stop=first,
                                skip_group_check=True,
                            )

                if t > 0:
                    hT_hi_prev = stpool.tile(
                        [128, 2 * 128], bf16, tag="hThi", name=f"hThi_{t - 1}"
                    )

                    def rec_cis(gp, wcol0, ncol0, nlen, cis, ob_shift=0):
                        for ci in cis:
                            lhs = hT_lhs(hT_lo_prev, hT_hi_prev, ci)
                            for half in range(2):
                                base = wcol0 + 512 * half + ncol0 + ob_shift
                                nc.tensor.matmul(
                                    out=gp[64 * half : 64 * (half + 1), ncol0 : ncol0 + nlen],
                                    lhsT=lhs,
                                    rhs=wh_sb[ci][:, base : base + nlen],
                                    start=False,
                                    stop=(ci == 7),
                                    skip_group_check=True,
                                )

                    LO, HI = (0, 4, 1, 5), (2, 6, 3, 7)
                    xb(gp_f, 2 * HH, 3 * HH)
                    rec_cis(gp_f, 2 * HIDDEN, 0, HQ, LO)
                    rec_cis(gp_f, 2 * HIDDEN, HQ, HQ, LO)
                    # deferred hi-transposes of the previous step's h
                    for q in range(2):
                        nc.tensor.transpose(
                            out=tp_hi[:, 128 * q : 128 * (q + 1)],
                            in_=h_prev[2 + q][:],
                            identity=ident[:],
                        )
                    nc.vector.tensor_copy(hT_hi_prev[:], tp_hi[:, 0:256])
                    xb(gp_g, HH, 2 * HH)
                    rec_cis(gp_g, HIDDEN, 0, HQ, LO)
                    rec_cis(gp_f, 2 * HIDDEN, 0, HQ, HI)
                    rec_cis(gp_f, 2 * HIDDEN, HQ, HQ, HI)
                    rec_cis(gp_g, HIDDEN, 0, HQ, HI)
                    rec_cis(gp_g, HIDDEN, HQ, HQ, CI_ORDER)
                    xb(gp_ia, 0, HQ)
                    xb(gp_ib, HQ, HH)
                    rec_cis(gp_ia, 0, 0, HQ, CI_ORDER)
                    rec_cis(gp_ib, 0, 0, HQ, CI_ORDER, ob_shift=HQ)
                    xb(gp_oa, 3 * HH, 3 * HH + HQ)
                    rec_cis(gp_oa, 3 * HIDDEN, 0, HQ, CI_ORDER)
                    xb(gp_ob, 3 * HH + HQ, 4 * HH)
                    rec_cis(gp_ob, 3 * HIDDEN, 0, HQ, CI_ORDER, ob_shift=HQ)
                else:
                    xb(gp_f, 2 * HH, 3 * HH)
                    xb(gp_g, HH, 2 * HH)
                    xb(gp_ia, 0, HQ)
                    xb(gp_ib, HQ, HH)
                    xb(gp_oa, 3 * HH, 3 * HH + HQ)
                    xb(gp_ob, 3 * HH + HQ, 4 * HH)

                # ---- epilogue for step t (chunks a=[0:256], b=[256:512]) ----
                sl = [slice(0, HQ), slice(HQ, HH)]
                def at(nm, k, dt=f32, pool=None):
                    pool = pool or apool
                    return pool.tile([128, HQ], dt, tag=f"{nm}{k}", name=f"{nm}{k}_{t}")
                sigi = [at("sigi", k) for k in range(2)]
                tanhg = [at("tanhg", k) for k in range(2)]
                sigf = [at("sigf", k) for k in range(2)]
                tanhc = [at("tanhc", k) for k in range(2)]
                sigo = [at("sigo", k) for k in range(2)]
                m1 = [at("m1", k) for k in range(2)]
                cmul = [at("cmul", k) for k in range(2)]
                c_new = [at("c", k, pool=stpool) for k in range(2)]
                hq = [
                    apool.tile([128, 128], bf16, tag=f"hq{q}", name=f"hq{q}_{t}")
                    for q in range(4)
                ]

                # ACT queue (1.2 GHz): f, g, i chunk acts as banks complete,
                # then tanhc_a, sigo_a, tanhc_b, sigo_b
                for k in range(2):
                    nc.scalar.activation(
                        sigf[k][:], gp_f[:, sl[k]], AF.Sigmoid, bias=1.0
                    )
                for k in range(2):
                    nc.scalar.activation(tanhg[k][:], gp_g[:, sl[k]], AF.Tanh)
                gpi = [gp_ia, gp_ib]
                for k in range(2):
                    nc.scalar.activation(sigi[k][:], gpi[k][:, 0:HQ], AF.Sigmoid)
                # GPS queue (slow ALU, SBUF-only): cmul has slack mid-chain
                for k in range(2):
                    nc.gpsimd.tensor_mul(cmul[k][:], sigf[k][:], c_prev[k][:])
                # DVE queue: m1_a, cnew_a, m1_b, cnew_b (cnew_a asap)
                nc.vector.tensor_mul(m1[0][:], sigi[0][:], tanhg[0][:])
                nc.vector.tensor_add(c_new[0][:], cmul[0][:], m1[0][:])
                nc.vector.tensor_mul(m1[1][:], sigi[1][:], tanhg[1][:])
                nc.vector.tensor_add(c_new[1][:], cmul[1][:], m1[1][:])
                # ACT tail.  tanhc_b carries an artificial zero-bias dep on
                # sigo_a: the scheduler's CoreSim cost model runs the PE ~2x
                # slower than reality (no column-tile concurrency), so without
                # the dep it enqueues tanhc_b before sigo_a in the strict ACT
                # FIFO and sigo_a (+the whole h/transpose/cast tail) blocks
                # ~1.5us behind tanhc_b's slow c-chain inputs.
                zb = apool.tile([128, 1], f32, tag="zb", name=f"zb_{t}")
                nc.scalar.activation(tanhc[0][:], c_new[0][:], AF.Tanh)
                nc.scalar.activation(sigo[0][:], gp_oa[:, 0:HQ], AF.Sigmoid)
                nc.vector.tensor_scalar_mul(zb[:], sigo[0][:, 0:1], 0.0)
                nc.scalar.activation(tanhc[1][:], c_new[1][:], AF.Tanh, bias=zb[:])
                nc.scalar.activation(sigo[1][:], gp_ob[:, 0:HQ], AF.Sigmoid)
                # DVE tail: h quarters q1/q2 feed this iteration's T0/T1
                for q in range(2):
                    k, c = divmod(128 * q, HQ)
                    nc.vector.tensor_mul(
                        hq[q][:], sigo[k][:, c : c + 128], tanhc[k][:, c : c + 128]
                    )

                # ---- h_a -> h^T low blocks (T0/T1) + cast ----
                hT_lo = stpool.tile([128, 2 * 128], bf16, tag="hTlo", name=f"hTlo_{t}")
                nc.tensor.transpose(
                    out=tp_lo[:, 0:128], in_=hq[0][:], identity=ident[:]
                )
                nc.tensor.transpose(
                    out=tp_lo[:, 128:256], in_=hq[1][:], identity=ident[:]
                )
                nc.vector.tensor_copy(hT_lo[:], tp_lo[:, 0:256])
                # h quarters q3/q4 on gpsimd (their transposes run early in
                # the next iteration; keeping them off the DVE queue keeps the
                # scheduler from displacing cast_lo)
                nc.gpsimd.tensor_mul(
                    hq[2][:], sigo[1][:, 0:128], tanhc[1][:, 0:128]
                )
                nc.gpsimd.tensor_mul(
                    hq[3][:], sigo[1][:, 128:256], tanhc[1][:, 128:256]
                )

                c_prev = c_new
                hT_lo_prev = hT_lo
                h_prev = hq
                if not last:
                    xg_t = xg_n

            # ---- final step's hi transposes (deferred) ----
            hT_hi_prev = stpool.tile(
                [128, 2 * 128], bf16, tag="hThi", name="hThi_last"
            )
            for q in range(2):
                nc.tensor.transpose(
                    out=tp_hi[:, 128 * q : 128 * (q + 1)],
                    in_=h_prev[2 + q][:],
                    identity=ident[:],
                )
            nc.vector.tensor_copy(hT_hi_prev[:], tp_hi[:, 0:256])

            # ---- images encoder: out = images @ W_img + b_img ----
            # accumulator reuses gp_g's bank via the tag ring
            oip = pspool.tile([128, OUT // 2], f32, tag="gp_g", name="oip")
            nc.tensor.matmul(
                out=oip[:], lhsT=o2[:], rhs=bimg2[:],
                start=True, stop=False, skip_group_check=True,
            )
            for ci in range(16):
                lhs = imT_sl(ci)
                for half in range(2):
                    nc.tensor.matmul(
                        out=oip[64 * half : 64 * (half + 1), :],
                        lhsT=lhs,
                        rhs=wimg_sb[ci][:, 512 * half : 512 * (half + 1)],
                        start=False, stop=(ci == 15), skip_group_check=True,
                    )
            oimg_sb = opool.tile([128, OUT // 2], f32, tag="oimg")
            nc.vector.tensor_copy(oimg_sb[:], oip[:])
            nc.sync.dma_start(oimg_d[:], oimg_sb[:])

            # ---- hidden encoder: out = h @ W_hid + b_hid ----
            # reuse gp_f's bank (tag ring, bufs=1 -> same memory, WAR-tracked)
            ohp = pspool.tile([128, HH], f32, tag="gp_f", name="ohp")
            nc.tensor.matmul(
                out=ohp[:], lhsT=o2[:], rhs=bhid2[:],
                start=True, stop=False, skip_group_check=True,
            )
            for ci in range(8):
                lhs = hT_lhs(hT_lo_prev, hT_hi_prev, ci)
                for half in range(2):
                    nc.tensor.matmul(
                        out=ohp[64 * half : 64 * (half + 1), :],
                        lhsT=lhs,
                        rhs=whid_sb[ci][:, 512 * half : 512 * (half + 1)],
                        start=False, stop=(ci == 7), skip_group_check=True,
                    )
            ohid_sb = opool.tile([128, OUT // 2], f32, tag="ohid")
            nc.vector.tensor_copy(ohid_sb[:], ohp[:])
            nc.sync.dma_start(ohid_d[:], ohid_sb[:])

    nc.compile()
    return nc


def _host_prep(images, embed_table, W_cell, b_cell, W_img, b_img, W_hid, b_hid,
               message):
    """Builds the per-core input maps (all host-side preprocessing)."""
    from ml_dtypes import bfloat16

    W_x = W_cell[:EMB]          # [512, 4096]
    W_h = np.ascontiguousarray(W_cell[EMB:]).astype(bfloat16)  # [1024, 4096]

    M2 = embed_table.astype(np.float32) @ W_x + b_cell  # [1024, 4096]
    # (the f-gate +1.0 is applied as an activation bias on-device)
    # row 2v+h = [i_h, g_h, f_h, o_h] halves of vocab row v
    M2p = np.ascontiguousarray(
        M2.reshape(VOCAB, 4, 2, HH).transpose(0, 2, 1, 3).reshape(2 * VOCAB, 4 * HH)
    ).astype(bfloat16)

    sfull = np.zeros((2 * BS, 2 * BS), np.float32)
    for m in range(BS):
        sfull[2 * m, m] = 1.0
        sfull[2 * m + 1, BS + m] = 1.0
    sfull = sfull.astype(bfloat16)

    ident = np.eye(128, dtype=np.float32)

    o2 = np.zeros((2, 128), np.float32)
    o2[0, 0:64] = 1.0
    o2[1, 64:128] = 1.0

    W_img_b = W_img.astype(bfloat16)
    W_hid_b = W_hid.astype(bfloat16)
    bimg2 = np.stack([b_img[: OUT // 2], b_img[OUT // 2 :]]).astype(np.float32)
    bhid2 = np.stack([b_hid[: OUT // 2], b_hid[OUT // 2 :]]).astype(np.float32)

    in_maps = []
    for core in range(NCORES):
        slc = slice(core * BS, (core + 1) * BS)
        msg = message[slc]  # [64, T] int32
        msg2 = np.empty((2 * BS, T), np.int32)
        msg2[0::2] = 2 * msg
        msg2[1::2] = 2 * msg + 1
        in_maps.append(
            {
                "m2p": M2p,
                "wh": W_h,
                "msg2": msg2,
                "sfull": sfull,
                "ident": ident.astype(bfloat16),
                "identf": ident,
                "imgs": np.concatenate(
                    [images[slc, : D_IMG // 2], images[slc, D_IMG // 2 :]], axis=0
                ),
                "wimg": W_img_b,
                "whid": W_hid_b,
                "o2": o2,
                "bimg2": bimg2,
                "bhid2": bhid2,
            }
        )
    return in_maps


def kernel(images, embed_table, W_cell, b_cell, W_img, b_img, W_hid, b_hid,
           message):
    import sys
    if "/opt/trn_rl_repo" not in sys.path:
        sys.path.insert(0, "/opt/trn_rl_repo")
    from concourse.bass_utils import run_bass_kernel_spmd

    images = np.asarray(images, np.float32)
    embed_table = np.asarray(embed_table, np.float32)
    W_cell = np.asarray(W_cell, np.float32)
    b_cell = np.asarray(b_cell, np.float32)
    W_img = np.asarray(W_img, np.float32)
    b_img = np.asarray(b_img, np.float32)
    W_hid = np.asarray(W_hid, np.float32)
    b_hid = np.asarray(b_hid, np.float32)
    message = np.asarray(message, np.int32)

    n_steps = T
    if "nc" not in _CACHE or _CACHE.get("n_steps") != n_steps:
        _CACHE["nc"] = _build_nc(n_steps)
        _CACHE["n_steps"] = n_steps
    nc = _CACHE["nc"]

    in_maps = _host_prep(
        images, embed_table, W_cell, b_cell, W_img, b_img, W_hid, b_hid, message
    )
    res = run_bass_kernel_spmd(nc, in_maps, core_ids=list(range(NCORES)))
    results = res.results

    images_encoded = np.empty((B, OUT), np.float32)
    hidden_encoded = np.empty((B, OUT), np.float32)
    for core in range(NCORES):
        slc = slice(core * BS, (core + 1) * BS)
        oi = results[core]["oimg"]
        oh = results[core]["ohid"]
        images_encoded[slc, : OUT // 2] = oi[0:64]
        images_encoded[slc, OUT // 2 :] = oi[64:128]
        hidden_encoded[slc, : OUT // 2] = oh[0:64]
        hidden_encoded[slc, OUT // 2 :] = oh[64:128]
    return images_encoded, hidden_encoded
